# revision 16
# baseline (speedup 1.0000x reference)
"""Bass/Trainium2 kernel for nn_BitGatConv (GAT-style message passing).

Self-contained: takes full inputs, shards edges by destination window across
8 NeuronCores (SPMD, one program), returns the full [N, HC] output.

Algorithm (per core, rotated node ids so all cores run the same program):
  Phase A (build): h = nodes_ft @ W, att_j = nodes_ft @ (W@A2),
    att_i = nodes_ft @ (W@A1); store bf16 tables
      hj_table [N_PAD, 128]  rows = [h | att_j]
      ao_table [NSHARD+1, 128] rows = [att_i | onehot64(node mod 64)]
      (row NSHARD = sentinel: att_i = -1e4 so exp()==0 for pad edges)
  Phase B (edges): for each 128-edge bin, gather hj rows by src and ao rows
    by local tgt; s = att_i + att_j; l = max(0.2*s, s); x = exp(l);
    payload = [x*h | x]; one-hot matmul accumulates [numer | denom] into a
    per-64-node-window PSUM tile (K bins per window, K uniform).
    No segment-max subtraction: logits are bounded (~|s|<10) so exp is safe,
    and softmax is shift-free identical.
  Phase C (flush): out = numer / (denom + 1e-16) + bias.
"""

import math
import os
import sys
from contextlib import ExitStack

import numpy as np

for _p in ("/opt/trn_rl_repo",):
    if _p not in sys.path:
        sys.path.insert(0, _p)

import ml_dtypes  # noqa: E402

BF16_NP = ml_dtypes.bfloat16

# ---------------------------------------------------------------------------
# Problem constants (hardcoded per contest rules)
N_NODES = 50000
N_EDGES = 800000
IN_CH = 128
HC = 64
NEG_SLOPE = 0.2
N_CORES = 8
W_WIN = 64  # nodes per scatter window (one-hot width)
SENT_ATT = -10000.0


def _cfg(n_nodes, n_edges, n_cores=N_CORES, w=W_WIN):
    nw = math.ceil(n_nodes / w)
    npc = math.ceil(nw / n_cores)  # windows per core
    if npc % 2 == 1:
        npc += 1  # need even (flush in pairs)
    n_pad = n_cores * npc * w
    nshard = npc * w
    # group_nw: windows per gather-group (batch for gathers/DVE)
    group_nw = 1
    for cand in (7, 6, 5, 4, 8, 3, 2):
        if npc % cand == 0:
            group_nw = cand
            break
    return dict(
        N=n_nodes, E=n_edges, NC=n_cores, W=w, NPC=npc,
        N_PAD=n_pad, NSHARD=nshard, GROUP_NW=group_nw,
        T_TILES=n_pad // 128, SHARD_TILES=nshard // 128,
    )


def _prep(inputs, cfg):
    """Host-side preprocessing: shard + pad + index building (numpy only)."""
    N, E, NC, W = cfg["N"], cfg["E"], cfg["NC"], cfg["W"]
    NPC, N_PAD, NSHARD = cfg["NPC"], cfg["N_PAD"], cfg["NSHARD"]

    nodes_ft = np.asarray(inputs["nodes_ft"], dtype=np.float32)
    adj = np.asarray(inputs["adj_list"])
    weight = np.asarray(inputs["weight"], dtype=np.float32)
    a1 = np.asarray(inputs["att_layer_1"], dtype=np.float32)
    a2 = np.asarray(inputs["att_layer_2"], dtype=np.float32)
    bias = np.asarray(inputs["bias"], dtype=np.float32)

    tgt = adj[0].astype(np.int64)
    src = adj[1].astype(np.int64)

    win = tgt // W
    core = win // NPC
    wloc = win % NPC
    GW = cfg["GROUP_NW"]
    HL = N_PAD // 2  # hj table split point (int16 index reach)

    src_rot = (src - core * NSHARD) % N_PAD
    half = (src_rot >= HL).astype(np.int64)  # 0 = lo table, 1 = hi table

    grp = win * 2 + half
    cnt2 = np.bincount(grp, minlength=NC * NPC * 2)
    KL = max(1, int(math.ceil(cnt2[0::2].max() / 128.0)))
    KH = max(1, int(math.ceil(cnt2[1::2].max() / 128.0)))
    K = KL + KH
    B = NPC * K  # bins per core
    NB = GW * K  # bins per gather group
    ngroups = NPC // GW

    order = np.argsort(grp, kind="stable")
    starts = np.zeros(NC * NPC * 2 + 1, dtype=np.int64)
    starts[1:] = np.cumsum(cnt2)
    rank = np.arange(E, dtype=np.int64) - starts[grp[order]]

    eo = order
    c_e = core[eo]
    wl = wloc[eo]
    g_e = wl // GW
    wlg = wl % GW
    h_e = half[eo]
    j_e = rank // 128
    p_e = rank % 128
    # bin index within core: group-major, [GW windows' lo bins | GW hi bins]
    b_e = g_e * NB + np.where(
        h_e == 0, wlg * KL + j_e, GW * KL + wlg * KH + j_e)

    # int16 idx streams in dma_gather wrapped layout (idx i -> [i%16, i//16])
    def wrap16(stream2d):
        # stream2d: [NC, L] -> [NC, 128, L//16]
        ncc, L = stream2d.shape
        w = stream2d.reshape(ncc, L // 16, 16).transpose(0, 2, 1)
        return np.ascontiguousarray(np.tile(w, (1, 8, 1)))

    ao_s = np.full((NC, B * 128), NSHARD, dtype=np.int16)
    ao_s[c_e, b_e * 128 + p_e] = (tgt[eo] - c_e * NSHARD).astype(np.int16)

    # lo/hi bin serial numbers within core (for the per-half gather streams)
    lob_e = g_e * (GW * KL) + wlg * KL + j_e
    hib_e = g_e * (GW * KH) + wlg * KH + j_e
    lo_s = np.zeros((NC, NPC * KL * 128), dtype=np.int16)
    hi_s = np.zeros((NC, NPC * KH * 128), dtype=np.int16)
    m0 = h_e == 0
    lo_s[c_e[m0], lob_e[m0] * 128 + p_e[m0]] = src_rot[eo][m0].astype(np.int16)
    m1 = ~m0
    hi_s[c_e[m1], hib_e[m1] * 128 + p_e[m1]] = (
        src_rot[eo][m1] - HL).astype(np.int16)

    ao_idx = wrap16(ao_s)
    lo_idx = wrap16(lo_s)
    hi_idx = wrap16(hi_s)

    # rotated, transposed, padded node features (bf16)
    base = np.zeros((IN_CH, N_PAD), dtype=np.float32)
    base[:, :N] = nodes_ft.T

    wh = weight.astype(BF16_NP)
    wi = (weight @ a1).astype(BF16_NP)
    wj = (weight @ a2).astype(BF16_NP)

    oh = np.zeros((NSHARD + 1, HC), dtype=np.float32)
    oh[np.arange(NSHARD), np.arange(NSHARD) % W] = 1.0
    # wide windows (W < HC unused cols stay 0); sentinel points at slot 0
    oh[NSHARD, 0] = 1.0
    oh = oh.astype(BF16_NP)

    sent_row = np.full((1, HC), SENT_ATT, dtype=np.float32).astype(BF16_NP)

    npair = NPC // 2
    bias_full = np.tile(bias[None, :], (128, npair)).astype(np.float32)

    in_maps = []
    for c in range(NC):
        nftT = np.ascontiguousarray(np.roll(base, -c * NSHARD, axis=1))
        in_maps.append({
            "nodes_ftT": nftT.astype(BF16_NP),
            "wh": wh, "wi": wi, "wj": wj,
            "onehot_const": oh,
            "sent_row": sent_row,
            "lo_idx": lo_idx[c],
            "hi_idx": hi_idx[c],
            "ao_idx": ao_idx[c],
            "bias_bc": bias_full,
        })
    meta = dict(K=K, KL=KL, KH=KH, B=B)
    return in_maps, meta


def _build_program(cfg, K, KL, KH, debug_dump=False):
    import concourse.bacc as bacc
    import concourse.bass as bass
    import concourse.mybir as mybir
    import concourse.tile as tile

    BF16 = mybir.dt.bfloat16
    F32 = mybir.dt.float32
    I32 = mybir.dt.int32
    ALU = mybir.AluOpType
    ACT = mybir.ActivationFunctionType

    NPC, N_PAD, NSHARD = cfg["NPC"], cfg["N_PAD"], cfg["NSHARD"]
    T_TILES, SHARD_TILES = cfg["T_TILES"], cfg["SHARD_TILES"]
    GROUP_NW = cfg["GROUP_NW"]
    assert K == KL + KH
    B = NPC * K
    NB = GROUP_NW * K          # bins per gather group
    NBL = GROUP_NW * KL        # lo bins per group
    NBH = GROUP_NW * KH
    NGROUPS = NPC // GROUP_NW
    NPAIR = NPC // 2
    HL = N_PAD // 2

    nc = bacc.Bacc("TRN2", target_bir_lowering=False, debug=False)

    nodes_ftT = nc.dram_tensor("nodes_ftT", [IN_CH, N_PAD], BF16, kind="ExternalInput")
    wh_d = nc.dram_tensor("wh", [IN_CH, HC], BF16, kind="ExternalInput")
    wi_d = nc.dram_tensor("wi", [IN_CH, HC], BF16, kind="ExternalInput")
    wj_d = nc.dram_tensor("wj", [IN_CH, HC], BF16, kind="ExternalInput")
    oh_d = nc.dram_tensor("onehot_const", [NSHARD + 1, HC], BF16, kind="ExternalInput")
    sent_d = nc.dram_tensor("sent_row", [1, HC], BF16, kind="ExternalInput")
    I16 = mybir.dt.int16
    loidx_d = nc.dram_tensor("lo_idx", [128, NPC * KL * 8], I16, kind="ExternalInput")
    hiidx_d = nc.dram_tensor("hi_idx", [128, NPC * KH * 8], I16, kind="ExternalInput")
    aoidx_d = nc.dram_tensor("ao_idx", [128, NPC * K * 8], I16, kind="ExternalInput")
    bias_d = nc.dram_tensor("bias_bc", [128, NPAIR * HC], F32, kind="ExternalInput")
    out_d = nc.dram_tensor("out", [NSHARD, HC], F32, kind="ExternalOutput")

    hj_table = nc.dram_tensor("hj_table", [N_PAD, 2 * HC], BF16, kind="Internal")
    ao_table = nc.dram_tensor("ao_table", [NSHARD + 1, 2 * HC], BF16, kind="Internal")

    with tile.TileContext(nc) as tc, ExitStack() as ctx:
        const_pool = ctx.enter_context(tc.tile_pool(name="const", bufs=1))
        b_in = ctx.enter_context(tc.tile_pool(name="b_in", bufs=4))
        b_ps = ctx.enter_context(tc.tile_pool(name="b_ps", bufs=2, space="PSUM"))
        b_st = ctx.enter_context(tc.tile_pool(name="b_st", bufs=4))

        wh_sb = const_pool.tile([IN_CH, HC], BF16)
        nc.sync.dma_start(wh_sb[:], wh_d[:])
        wi_sb = const_pool.tile([IN_CH, HC], BF16)
        nc.sync.dma_start(wi_sb[:], wi_d[:])
        wj_sb = const_pool.tile([IN_CH, HC], BF16)
        nc.sync.dma_start(wj_sb[:], wj_d[:])
        bias_sb = const_pool.tile([128, NPAIR * HC], F32)
        nc.sync.dma_start(bias_sb[:], bias_d[:])

        # constant halves of ao_table (DRAM->DRAM)
        nc.sync.dma_start(ao_table[:, HC:2 * HC], oh_d[:])
        nc.sync.dma_start(ao_table[NSHARD:NSHARD + 1, 0:HC], sent_d[:])

        # ---- Phase A: build tables (full N_PAD, replicated on every core)
        for t in range(T_TILES):
            nf = b_in.tile([128, 128], BF16)
            nc.sync.dma_start(nf[:], nodes_ftT[:, 128 * t:128 * (t + 1)])
            ps = b_ps.tile([128, 3 * HC], F32)
            nc.tensor.matmul(ps[:, 0:HC], nf[:], wh_sb[:], start=True, stop=False)
            nc.tensor.matmul(ps[:, HC:2 * HC], nf[:], wj_sb[:], start=False, stop=False)
            nc.tensor.matmul(ps[:, 2 * HC:3 * HC], nf[:], wi_sb[:], start=False, stop=True)
            st = b_st.tile([128, 2 * HC], BF16)
            if t % 2 == 0:
                nc.vector.tensor_copy(st[:], ps[:, 0:2 * HC])
            else:
                nc.scalar.copy(st[:], ps[:, 0:2 * HC])
            nc.sync.dma_start(hj_table[128 * t:128 * (t + 1), :], st[:])
            if t < SHARD_TILES:
                sa = b_st.tile([128, HC], BF16, tag="sa")
                if t % 2 == 0:
                    nc.scalar.copy(sa[:], ps[:, 2 * HC:3 * HC])
                else:
                    nc.vector.tensor_copy(sa[:], ps[:, 2 * HC:3 * HC])
                nc.sync.dma_start(ao_table[128 * t:128 * (t + 1), 0:HC], sa[:])

        tc.strict_bb_all_engine_barrier()

        # ---- Phase B: edge processing
        idx_pool = ctx.enter_context(tc.tile_pool(name="idx", bufs=4))
        g_pool = ctx.enter_context(tc.tile_pool(name="gp", bufs=2))
        ao_pool = ctx.enter_context(tc.tile_pool(name="aop", bufs=2))
        s_pool = ctx.enter_context(tc.tile_pool(name="sp", bufs=2))
        mm_ps = ctx.enter_context(tc.tile_pool(name="mmps", bufs=4, space="PSUM"))
        fl_pool = ctx.enter_context(tc.tile_pool(name="fl", bufs=1))

        stage_n = fl_pool.tile([128, NPAIR * HC], F32)
        stage_d = fl_pool.tile([128, NPAIR * HC], F32)

        pair_tiles = {}
        for g in range(NGROUPS):
            sl = idx_pool.tile([128, NBL * 8], I16, tag="sl")
            nc.sync.dma_start(sl[:], loidx_d[:, g * NBL * 8:(g + 1) * NBL * 8])
            sh = idx_pool.tile([128, NBH * 8], I16, tag="sh")
            nc.sync.dma_start(sh[:], hiidx_d[:, g * NBH * 8:(g + 1) * NBH * 8])
            ai = idx_pool.tile([128, NB * 8], I16, tag="ai")
            nc.sync.dma_start(ai[:], aoidx_d[:, g * NB * 8:(g + 1) * NB * 8])

            G = g_pool.tile([128, NB, 2 * HC], BF16, tag="G")
            nc.gpsimd.dma_gather(
                out_ap=G[:, 0:NBL, :], in_ap=hj_table[0:HL, :],
                idxs_ap=sl[:], num_idxs=NBL * 128, num_idxs_reg=NBL * 128,
                elem_size=2 * HC, queue_num=0, single_packet=False,
            )
            nc.gpsimd.dma_gather(
                out_ap=G[:, NBL:NB, :], in_ap=hj_table[HL:N_PAD, :],
                idxs_ap=sh[:], num_idxs=NBH * 128, num_idxs_reg=NBH * 128,
                elem_size=2 * HC, queue_num=0, single_packet=False,
            )
            AOt = ao_pool.tile([128, NB, 2 * HC], BF16, tag="AO")
            nc.gpsimd.dma_gather(
                out_ap=AOt[:], in_ap=ao_table[:],
                idxs_ap=ai[:], num_idxs=NB * 128, num_idxs_reg=NB * 128,
                elem_size=2 * HC, queue_num=0, single_packet=False,
            )

            S = s_pool.tile([128, NB, HC], BF16, tag="S")
            # s = att_j + att_i
            nc.vector.tensor_tensor(
                out=S[:], in0=G[:, :, HC:2 * HC], in1=AOt[:, :, 0:HC], op=ALU.add)
            # l = max(0.2*s, s)  (leaky relu)
            nc.vector.scalar_tensor_tensor(
                out=S[:], in0=S[:], scalar=NEG_SLOPE, in1=S[:],
                op0=ALU.mult, op1=ALU.max)
            # x = exp(l) -> overwrite att_j half of G
            nc.scalar.activation(G[:, :, HC:2 * HC], S[:], ACT.Exp)
            # y = h * x -> overwrite h half of G
            nc.vector.tensor_tensor(
                out=G[:, :, 0:HC], in0=G[:, :, 0:HC], in1=G[:, :, HC:2 * HC],
                op=ALU.mult)

            for bl in range(NB):
                if bl < NBL:
                    w = g * GROUP_NW + bl // KL
                    j = bl % KL
                else:
                    l2 = bl - NBL
                    w = g * GROUP_NW + l2 // KH
                    j = KL + l2 % KH
                pr, half = w // 2, w % 2
                if j == 0 and half == 0:
                    pair_tiles[pr] = mm_ps.tile(
                        [128, 2 * HC], F32, tag="pp", name=f"pp{pr}")
                ps_t = pair_tiles[pr]
                nc.tensor.matmul(
                    ps_t[HC * half:HC * half + HC, :],
                    AOt[:, bl, HC:2 * HC],
                    G[:, bl, :],
                    start=(j == 0), stop=(j == K - 1),
                    tile_position=(0, HC * half),
                    skip_group_check=True,
                )
                if j == K - 1 and half == 1:
                    nc.vector.tensor_copy(
                        stage_n[:, HC * pr:HC * (pr + 1)], ps_t[:, 0:HC])
                    nc.vector.tensor_copy(
                        stage_d[:, HC * pr:HC * (pr + 1)], ps_t[:, HC:2 * HC])
                    del pair_tiles[pr]

        # ---- Phase C: out = numer / (denom + eps) + bias
        nc.vector.tensor_scalar_add(stage_d[:], stage_d[:], 1e-16)
        lnd = fl_pool.tile([128, NPAIR * HC], F32)
        nc.scalar.activation(lnd[:], stage_d[:], ACT.Ln)
        nc.scalar.activation(lnd[:], lnd[:], ACT.Exp, scale=-1.0)
        nc.vector.tensor_tensor(out=stage_n[:], in0=stage_n[:], in1=lnd[:], op=ALU.mult)
        nc.vector.tensor_tensor(out=stage_n[:], in0=stage_n[:], in1=bias_sb[:], op=ALU.add)

        out_view = out_d[:].rearrange("(pr p) c -> p pr c", p=128)
        st_view = stage_n[:].rearrange("p (pr c) -> p pr c", c=HC)
        nc.sync.dma_start(out_view, st_view)

        if debug_dump:
            dump_hj = nc.dram_tensor("dump_hj", [N_PAD, 2 * HC], BF16,
                                     kind="ExternalOutput")
            dump_ao = nc.dram_tensor("dump_ao", [NSHARD + 1, 2 * HC], BF16,
                                     kind="ExternalOutput")
            dump_sd = nc.dram_tensor("dump_sd", [128, NPAIR * HC], F32,
                                     kind="ExternalOutput")
            dump_g = nc.dram_tensor("dump_g", [128, NB * 2 * HC], BF16,
                                    kind="ExternalOutput")
            dump_aot = nc.dram_tensor("dump_aot", [128, NB * 2 * HC], BF16,
                                      kind="ExternalOutput")
            tc.strict_bb_all_engine_barrier()
            nc.sync.dma_start(dump_hj[:], hj_table[:])
            nc.sync.dma_start(dump_ao[:], ao_table[:])
            nc.sync.dma_start(dump_sd[:], stage_d[:])
            nc.sync.dma_start(dump_g[:], G[:].rearrange("p a b -> p (a b)"))
            nc.sync.dma_start(dump_aot[:], AOt[:].rearrange("p a b -> p (a b)"))

    nc.compile()
    return nc


def kernel(**inputs):
    cfg = _cfg(N_NODES, N_EDGES)
    in_maps, meta = _prep(inputs, cfg)
    nc = _build_program(cfg, meta["K"], meta["KL"], meta["KH"])

    from concourse import bass_utils
    res = bass_utils.run_bass_kernel_spmd(
        nc, in_maps, core_ids=list(range(cfg["NC"])),
        trace=bool(int(os.environ.get("GAT_TRACE", "0"))),
    )
    kernel.last_result = res  # stash for test harness (exec_time_ns etc.)
    kernel.last_ctx = (nc, in_maps, cfg)

    NSHARD = cfg["NSHARD"]
    out_full = np.zeros((cfg["NC"] * NSHARD, HC), dtype=np.float32)
    for c in range(cfg["NC"]):
        out_full[c * NSHARD:(c + 1) * NSHARD] = res.results[c]["out"]
    return out_full[:cfg["N"]]
